# revision 17
# baseline (speedup 1.0000x reference)
"""Trainium2 Bass kernel for the NeuralMemory (scatter_memory) problem.

Math summary (B=1, N=512, D=128, DEPTH=4):
  The per-token meta-gradients of the memory MLP are rank-1 per layer:
      grad_l(token s) = outer(x_l(s), delta_{l+1}(s))
  so the (n, depth, d, d) momentum/update scans collapse to a scalar,
  per-token-pair coefficient matrix C[t,s] (composition of the momentum and
  decay linear recurrences) applied attention-style:
      retrieved_l(t) = y_t @ W_l + sum_s C[t,s] * (y_t . x_l(s)) * g'_l(s)
  C^T is built exactly on-device with the hardware linear-recurrence scan
  (tensor_tensor_scan):  A^T[s,t]: state = am_t*state + [t==s]
                         C^T[s,t]: state = (1-decay_t)*state + A^T[s,t]
  All tensors live in transposed (d, n) layout so every matmul contracts on
  the partition dim. The full problem fits in SBUF; the program is replicated
  SPMD across the 8 cores (compute is tiny; replication avoids collectives).
"""

import numpy as np

D = 128
N = 512
DEPTH = 4
NCORES = 8
CH = 128          # s-chunk size
NCH = N // CH     # 4 chunks

# column offsets inside the single consolidated input tensor (128, ALLIN_W)
OFF_SEQT = 0                    # (128, 512)  seq^T
OFF_WQ = 512                    # (128, 128)
OFF_WK = 640
OFF_WV = 768
OFF_WM = 896                    # 4 x (128, 128)  W_mem layers
OFF_WMT = 1408                  # 4 x (128, 128)  W_mem layers transposed
OFF_ID = 1920                   # (128, 128) identity
OFF_WROWS = 2048                # (128, 96) W_step@+0, W_mom@+32, W_decay@+64
OFF_IZ = 2144                   # (128, 512) [I | 0] scan impulse
ALLIN_W = 2656

_cache = {}


def _build_program():
    import concourse.mybir as mybir
    from concourse import bacc
    from concourse.tile import TileContext

    f32 = mybir.dt.float32
    fp16 = mybir.dt.float16
    AF = mybir.ActivationFunctionType
    ALU = mybir.AluOpType

    nc = bacc.Bacc("TRN2")

    allin_d = nc.dram_tensor("allin", [D, ALLIN_W], fp16, kind="ExternalInput")
    outT_d = nc.dram_tensor("outT", [D, N], f32, kind="ExternalOutput")

    with TileContext(nc) as tc:
        with (
            tc.tile_pool(name="sb", bufs=1) as sb,
            tc.tile_pool(name="tmp", bufs=3) as tmp,
            tc.tile_pool(name="cst", bufs=4) as cstp,
            tc.tile_pool(name="gsb", bufs=1) as gsb,
            tc.tile_pool(name="ps_mm", bufs=4, space="PSUM") as ps_mm,
            tc.tile_pool(name="ps_acc", bufs=2, space="PSUM") as ps_acc,
            tc.tile_pool(name="ps_tp", bufs=2, space="PSUM") as ps_tp,
        ):
            def sbt(tag, shape=(D, N), dt=f32):
                return sb.tile(list(shape), dt, tag=tag, name=tag)

            # ---- single consolidated input DMA ----
            allin = sbt("allin", (D, ALLIN_W), dt=fp16)
            nc.sync.dma_start(out=allin, in_=allin_d[:, :])
            seqT = allin[:, OFF_SEQT:OFF_SEQT + N]
            wq = allin[:, OFF_WQ:OFF_WQ + D]
            wk = allin[:, OFF_WK:OFF_WK + D]
            wv = allin[:, OFF_WV:OFF_WV + D]
            wm = [allin[:, OFF_WM + D * l:OFF_WM + D * (l + 1)]
                  for l in range(DEPTH)]
            wmT = [allin[:, OFF_WMT + D * l:OFF_WMT + D * (l + 1)]
                   for l in range(DEPTH)]
            idm = allin[:, OFF_ID:OFF_ID + D]
            wrows = allin[:, OFF_WROWS:OFF_WROWS + 96]
            iz = allin[:, OFF_IZ:OFF_IZ + N]


            # ---- projections (transposed layout); copies on DVE ----
            def mm_to_sbuf(dst_tag, lhsT, rhs, m=D, dt=fp16):
                ps = ps_mm.tile([m, N], f32, tag="mm", name="mm")
                nc.tensor.matmul(ps, lhsT, rhs, start=True, stop=True)
                out = sbt(dst_tag, (m, N), dt=dt)
                nc.scalar.copy(out, ps)
                return out

            qT = mm_to_sbuf("qT", wq, seqT)
            x0 = mm_to_sbuf("x0", wk, seqT)
            vT = mm_to_sbuf("vT", wv, seqT, dt=f32)
            # one M=96 matmul: rows land at psum partitions 0 / 32 / 64
            ps_rows = ps_mm.tile([96, N], f32, tag="mm", name="mm")
            nc.tensor.matmul(ps_rows, wrows, seqT, start=True, stop=True)
            lrrow = sbt("lrrow", (1, N))
            nc.vector.tensor_scalar_mul(lrrow, ps_rows[0:1, :], -2.0 / D)
            amrow = sbt("amrow", (1, N))
            nc.vector.tensor_copy(amrow, ps_rows[32:33, :])
            # brow = 1 - sigmoid(dec) = 0.5 - 0.5*tanh(dec/2)
            throw = sbt("throw", (1, N))
            nc.scalar.activation(throw, ps_rows[64:65, :], AF.Tanh, scale=0.5)
            brow = sbt("brow", (1, N))
            nc.scalar.activation(brow, throw, AF.Copy, scale=-0.5, bias=0.5)

            # ---- broadcast rows along partitions (GPSIMD custom op) ----
            def bcast(dst_tag, row):
                out = sbt(dst_tag)
                nc.gpsimd.partition_broadcast(out, row)
                return out

            LRB = bcast("LRB", lrrow)   # (-2/D)*lr broadcast
            AMB = bcast("AMB", amrow)
            BB = bcast("BB", brow)

            # ---- scans: build A^T then C^T per s-chunk ----
            CT = [sbt(f"CT{k}") for k in range(NCH)]
            AT = [sbt(f"AT{k}") for k in range(NCH)]
            for k in range(NCH):
                t0 = CH * k
                if k > 0:
                    nc.gpsimd.memset(AT[k][:, 0:t0], 0.0)
                    nc.gpsimd.memset(CT[k][:, 0:t0], 0.0)
                nc.vector.tensor_tensor_scan(
                    AT[k][:, t0:N], AMB[:, t0:N], iz[:, 0:N - t0],
                    0.0, ALU.mult, ALU.add,
                )
                nc.vector.tensor_tensor_scan(
                    CT[k][:, t0:N], BB[:, t0:N], AT[k][:, t0:N],
                    0.0, ALU.mult, ALU.add,
                )

            # ---- store forward (X_l^T); H stashed to SBUF for Dsilu later ----
            X = [x0]
            h_ps = []
            Hs = [None]
            for l in range(DEPTH):
                ps = ps_mm.tile([D, N], f32, tag="mm", name="mm")
                nc.tensor.matmul(ps, wm[l], X[l], start=True, stop=True)
                h_ps.append(ps)
                if l < DEPTH - 1:
                    xl = sbt(f"x{l + 1}", dt=fp16)
                    nc.scalar.activation(xl, ps, AF.Silu)
                    X.append(xl)
                    hsb = sbt(f"h{l + 1}")
                    nc.vector.tensor_copy(hsb, ps)
                    Hs.append(hsb)
            # grouped Derivative_silu (one ACT table-set visit)
            SP = [None]
            for l in range(1, DEPTH):
                spl = sbt(f"sp{l}")
                nc.scalar.activation(spl, Hs[l], AF.Derivative_silu)
                SP.append(spl)

            # ---- backward deltas (scaled by -2/D*lr via LRB) ----
            Dl = [None] * (DEPTH + 1)
            d4a = tmp.tile([D, N], f32, tag="t", name="t")
            nc.vector.tensor_sub(d4a, h_ps[3], vT)
            d4 = sbt("d4", dt=fp16)
            nc.vector.tensor_mul(d4, d4a, LRB)
            Dl[4] = d4
            for l in range(DEPTH - 1, 0, -1):
                ps = ps_mm.tile([D, N], f32, tag="mm", name="mm")
                nc.tensor.matmul(ps, wmT[l], Dl[l + 1], start=True, stop=True)
                dl = sbt(f"d{l}", dt=fp16)
                nc.vector.tensor_mul(dl, ps, SP[l])
                Dl[l] = dl

            # ---- G'_l = delta'_{l+1} transposed to (s, j), per chunk ----
            G = [[gsb.tile([CH, D], fp16, tag=f"g{l}_{k}", name=f"g{l}_{k}")
                  for k in range(NCH)] for l in range(DEPTH)]
            for l in range(DEPTH):
                dsrc = Dl[l + 1]
                for k in range(NCH):
                    nc.sync.dma_start_transpose(G[l][k],
                                                dsrc[:, CH * k:CH * (k + 1)])

            # ---- retrieval ----
            Y = qT
            for l in range(DEPTH):
                cst = [None] * NCH
                for k in range(NCH - 1, -1, -1):
                    t0 = CH * k
                    ps_st = ps_mm.tile([CH, N], f32, tag="mm", name="mm")
                    nc.tensor.matmul(
                        ps_st[:, t0:N], X[l][:, CH * k:CH * (k + 1)],
                        Y[:, t0:N], start=True, stop=True,
                    )
                    c_t = cstp.tile([CH, N], fp16, tag="cst", name="cst")
                    nc.vector.tensor_mul(c_t[:, t0:N], ps_st[:, t0:N],
                                         CT[k][:, t0:N])
                    cst[k] = c_t
                racc = ps_acc.tile([D, N], f32, tag="racc", name="racc")
                nc.tensor.matmul(racc, wm[l], Y, start=True, stop=False)
                for k in range(NCH - 1, -1, -1):
                    t0 = CH * k
                    nc.tensor.matmul(
                        racc[:, t0:N], G[l][k], cst[k][:, t0:N],
                        start=False, stop=(k == 0),
                    )
                if l < DEPTH - 1:
                    ynext = sbt(f"y{l + 1}", dt=fp16)
                    nc.scalar.activation(ynext, racc, AF.Silu)
                    Y = ynext
                else:
                    outT = sbt("outT")
                    nc.vector.tensor_copy(outT, racc)

            nc.sync.dma_start(out=outT_d[:, :], in_=outT)

    return nc


def get_program():
    if "nc" not in _cache:
        nc = _build_program()
        nc.finalize()
        _cache["nc"] = nc
    return _cache["nc"]


def make_in_map(seq, W_mem, W_q, W_kv, W_mom, W_step, W_decay):
    seq = np.asarray(seq, dtype=np.float32)
    W_mem = np.asarray(W_mem, dtype=np.float32)
    W_kv = np.asarray(W_kv, dtype=np.float32)
    allin = np.zeros((D, ALLIN_W), dtype=np.float16)
    allin[:, OFF_SEQT:OFF_SEQT + N] = seq.reshape(N, D).T.astype(np.float16)
    allin[:, OFF_WQ:OFF_WQ + D] = np.asarray(W_q, dtype=np.float32)
    allin[:, OFF_WK:OFF_WK + D] = W_kv[:, :D]
    allin[:, OFF_WV:OFF_WV + D] = W_kv[:, D:]
    for l in range(DEPTH):
        allin[:, OFF_WM + D * l:OFF_WM + D * (l + 1)] = W_mem[l]
        allin[:, OFF_WMT + D * l:OFF_WMT + D * (l + 1)] = W_mem[l].T
    allin[:, OFF_ID:OFF_ID + D] = np.eye(D, dtype=np.float32)
    allin[:, OFF_WROWS + 0] = np.asarray(W_step, dtype=np.float32)[:, 0]
    allin[:, OFF_WROWS + 32] = np.asarray(W_mom, dtype=np.float32)[:, 0]
    allin[:, OFF_WROWS + 64] = np.asarray(W_decay, dtype=np.float32)[:, 0]
    allin[:, OFF_IZ:OFF_IZ + D] = np.eye(D, dtype=np.float32)
    return {"allin": allin}


def kernel(**inputs) -> np.ndarray:
    from concourse.bass_utils import run_bass_kernel_spmd

    nc = get_program()
    in_map = make_in_map(**inputs)
    in_maps = [in_map for _ in range(NCORES)]
    res = run_bass_kernel_spmd(nc, in_maps, list(range(NCORES)))
    outT = res.results[0]["outT"]
    return np.ascontiguousarray(outT.T).reshape(1, N, D).astype(np.float32)
